# revision 3
# baseline (speedup 1.0000x reference)
"""Multi-head attention (B=4, S=2048, D=1024, H=16, causal) on 8 trn2 cores.

Sharding: core c = (batch b = c//2, head-group g = c%2); 8 heads, 512 head
dims per core. fp16 matmul operands, fp32 PSUM accumulation.

v2 structure — key-tile-major ("stripe") attention order:
  phase 0: q-projection for ALL four 512-query blocks up front.
  stripe s (s=0..3): k/v projection for key tiles 4s..4s+3, then attention
  for every live query block j >= s against those key tiles. AV partials
  accumulate per (head, qblock) in PSUM within a stripe and are flushed to
  fp16 SBUF accumulators (ysum) between stripes. Query block s completes at
  stripe s: normalize (deferred 1/rowsum via ones-column + emat broadcast
  matmul) and its partial out-projection run there.

  This front-loads the exp() load (early key tiles hit all query blocks)
  so the ACT engine's heavy stripes overlap the remaining projection
  matmuls, instead of block-major order where the last block's exp has no
  concurrent PE work.

Score matmuls have K=64 (head dim) and are issued as row-group pairs: head
2m occupies PE rows 0:63, head 2m+1 rows 64:127 (tile_position auto-derived
from base partitions), letting the two heads' score matmuls execute
concurrently on disjoint PE quadrants.

Initial input DMA is spread across the three DMA queues (sync: xq,
scalar: weights, gpsimd: xk/xv) so the first projection matmul starts
~4us in rather than ~19us.
"""

import sys

if "/opt/trn_rl_repo" not in sys.path:
    sys.path.insert(0, "/opt/trn_rl_repo")

from contextlib import ExitStack

import numpy as np

import concourse.bacc as bacc
import concourse.mybir as mybir
import concourse.tile as tile
from concourse.bass_utils import run_bass_kernel_spmd

B, S, D = 4, 2048, 1024
H, DK = 16, 64
G = 2  # head groups (tensor parallel)
HPG = H // G  # 8 heads per core
HD = HPG * DK  # 512 head dims per core
NC = 8
P = 128
NT = S // P  # 16 key tiles of 128
NJ = S // 512  # 4 query blocks of 512
KC = D // P  # 8 d_model chunks
MC = HD // P  # 4 head-dim chunks (= head pairs)

F32 = mybir.dt.float32
DT = mybir.dt.float16
NPDT = np.float16
EXP = mybir.ActivationFunctionType.Exp

_CACHE = {}


def _emat():
    e = np.zeros((HPG, MC, P), dtype=NPDT)
    for c in range(MC):
        e[2 * c, c, 0:64] = 1.0
        e[2 * c + 1, c, 64:128] = 1.0
    return e


def _build():
    nc = bacc.Bacc("TRN2", target_bir_lowering=False, debug=False)

    # inputs pre-blocked host-side so every DMA line is >=8KB contiguous
    xqT = nc.dram_tensor("xqT", [NJ, P, KC, 512], DT, kind="ExternalInput")
    xkT = nc.dram_tensor("xkT", [NJ, P, KC, 512], DT, kind="ExternalInput")
    xvT = nc.dram_tensor("xvT", [NJ, P, KC, 512], DT, kind="ExternalInput")
    wqT = nc.dram_tensor("wqT", [P, KC, HD], DT, kind="ExternalInput")
    wkT = nc.dram_tensor("wkT", [P, KC, HD], DT, kind="ExternalInput")
    wvT = nc.dram_tensor("wvT", [P, KC, HD], DT, kind="ExternalInput")
    wpg = nc.dram_tensor("wpg", [P, MC, D], DT, kind="ExternalInput")
    ein = nc.dram_tensor("ein", [HPG, MC, P], DT, kind="ExternalInput")
    out = nc.dram_tensor("out", [NJ, 4, 2, P, 512], DT, kind="ExternalOutput")

    with tile.TileContext(nc) as tc, ExitStack() as ctx:
        persist = ctx.enter_context(tc.tile_pool(name="persist", bufs=1))

        qT = [persist.tile([P, S], DT, name=f"qT{m}", tag=f"qT{m}") for m in range(MC)]
        kT = [persist.tile([P, S], DT, name=f"kT{m}", tag=f"kT{m}") for m in range(MC)]
        vext = [
            persist.tile([P, HPG, 66], DT, name=f"vext{t}", tag=f"vext{t}")
            for t in range(NT)
        ]
        emat = persist.tile([HPG, MC, P], DT, name="emat", tag="emat")
        wp_sb = persist.tile([P, MC, D], DT, name="wp_sb", tag="wp_sb")
        wq_sb = persist.tile([P, KC, HD], DT, name="wq_sb", tag="wq_sb")
        wk_sb = persist.tile([P, KC, HD], DT, name="wk_sb", tag="wk_sb")
        wv_sb = persist.tile([P, KC, HD], DT, name="wv_sb", tag="wv_sb")
        # fp16 AV accumulators per query block: [65, head, 512]; row 64
        # carries the softmax denominator for all 8 heads side by side
        ysum = [
            persist.tile([65, HPG, 512], DT, name=f"ys{j}", tag=f"ys{j}")
            for j in range(NJ)
        ]

        # Critical-path loads first, split so the first matmuls can start
        # after ~512KB per queue: scalar carries wq halves then wk/wv;
        # sync carries xq block 0 halves then the rest of xq and wp.
        nc.scalar.dma_start(out=wq_sb[:], in_=wqT.ap())
        nc.gpsimd.dma_start(out=emat[:], in_=ein.ap())
        nc.scalar.dma_start(out=wk_sb[:], in_=wkT.ap())

        with tc.tile_pool(name="init", bufs=1) as initpool:
            onecol = initpool.tile([P, HPG], F32, name="onecol", tag="onecol")
            nc.vector.memset(onecol[:], 1.0)
            for t in range(NT):
                nc.vector.tensor_copy(
                    vext[t][:, :, 64:65],
                    onecol[:].rearrange("p (h o) -> p h o", o=1),
                )

        with (
            tc.tile_pool(name="psA", bufs=2, space="PSUM") as psA,
            tc.tile_pool(name="ps_s", bufs=2, space="PSUM") as ps_s,
            tc.tile_pool(name="ps_y", bufs=2, space="PSUM") as ps_y,
            tc.tile_pool(name="xqp", bufs=2) as xqpool,
            tc.tile_pool(name="xkp", bufs=2) as xkpool,
            tc.tile_pool(name="xvp", bufs=2) as xvpool,
            tc.tile_pool(name="attn", bufs=4) as attn_pool,
            tc.tile_pool(name="ypool", bufs=2) as ypool,
            tc.tile_pool(name="rpool", bufs=2) as rpool,
            tc.tile_pool(name="opool", bufs=3) as opool,
        ):
            def xq_dma(b):
                xt = xqpool.tile([P, KC, 512], DT, name="xq", tag="xq")
                nc.sync.dma_start(out=xt[:], in_=xqT.ap()[b])
                return xt

            def qproj_block(b, xt, ms=range(MC)):
                for m in ms:
                    pt = psA.tile([P, 512], F32, name="psA", tag="psA")
                    for kc in range(KC):
                        nc.tensor.matmul(
                            pt[:],
                            wq_sb[:, kc, m * P : (m + 1) * P],
                            xt[:, kc, :],
                            start=(kc == 0),
                            stop=(kc == KC - 1),
                        )
                    nc.vector.tensor_copy(
                        qT[m][:, b * 512 : (b + 1) * 512], pt[:]
                    )

            def kv_dma(s):
                xk = xkpool.tile([P, KC, 512], DT, name="xk", tag="xk")
                nc.gpsimd.dma_start(out=xk[:], in_=xkT.ap()[s])
                xv = xvpool.tile([P, KC, 512], DT, name="xv", tag="xv")
                nc.gpsimd.dma_start(out=xv[:], in_=xvT.ap()[s])
                return xk, xv

            def kproj_stripe(s, xk, ms=range(MC)):
                for m in ms:
                    pt = psA.tile([P, 512], F32, name="psK", tag="psA")
                    for kc in range(KC):
                        nc.tensor.matmul(
                            pt[:],
                            wk_sb[:, kc, m * P : (m + 1) * P],
                            xk[:, kc, :],
                            start=(kc == 0),
                            stop=(kc == KC - 1),
                        )
                    nc.vector.tensor_copy(
                        kT[m][:, s * 512 : (s + 1) * 512], pt[:]
                    )

            def vproj_stripe(s, xv):
                for t in range(4 * s, 4 * s + 4):
                    tt = t % 4
                    pv = psA.tile([P, 512], F32, name="psV", tag="psA")
                    for kc in range(KC):
                        nc.tensor.matmul(
                            pv[:],
                            xv[:, kc, tt * P : (tt + 1) * P],
                            wv_sb[:, kc, :],
                            start=(kc == 0),
                            stop=(kc == KC - 1),
                        )
                    nc.vector.tensor_copy(
                        vext[t][:, :, 0:64],
                        pv[:].rearrange("p (h d) -> p h d", h=HPG),
                    )

            def attn_stripe_block(s, j, hps=range(MC)):
                # attention of query block j against key tiles 4s..4s+3
                for hp in hps:
                    he, ho = 2 * hp, 2 * hp + 1
                    py_e = ps_y.tile([65, 512], F32, name="pye", tag="py")
                    py_o = ps_y.tile([65, 512], F32, name="pyo", tag="py")
                    for i in range(4 * s, 4 * s + 4):
                        d = 128 * i - 512 * j
                        tr = max(0, d)
                        pssc = ps_s.tile([P, 1024], F32, name="pssc", tag="pssc")
                        at = attn_pool.tile([P, 1024], DT, name="at", tag="at")
                        # two heads on disjoint PE row groups (concurrent)
                        for z, poff in ((0, 0), (1, 64)):
                            nc.tensor.matmul(
                                pssc[:, z * 512 + tr : (z + 1) * 512],
                                kT[hp][poff : poff + 64, i * P : (i + 1) * P],
                                qT[hp][
                                    poff : poff + 64,
                                    j * 512 + tr : (j + 1) * 512,
                                ],
                                start=True,
                                stop=True,
                            )
                        nc.scalar.activation(
                            out=at[:, tr:1024],
                            in_=pssc[:, tr:1024],
                            func=EXP,
                            scale=0.125,
                        )
                        if d >= 0:  # diagonal tile: causal mask both heads
                            for z in (0, 1):
                                nc.gpsimd.affine_select(
                                    out=at[:, z * 512 + tr : (z + 1) * 512],
                                    in_=at[:, z * 512 + tr : (z + 1) * 512],
                                    compare_op=mybir.AluOpType.is_ge,
                                    fill=0.0,
                                    base=tr - d,
                                    pattern=[[1, 512 - tr]],
                                    channel_multiplier=-1,
                                )
                        for z, h, py in ((0, he, py_e), (1, ho, py_o)):
                            nc.tensor.matmul(
                                py[:, tr:512],
                                vext[i][:, h, 0:65],
                                at[:, z * 512 + tr : (z + 1) * 512],
                                start=(i == 4 * s),
                                stop=(i == 4 * s + 3),
                            )
                    # flush stripe partials into fp16 accumulators
                    for h, py in ((he, py_e), (ho, py_o)):
                        if s == 0:
                            nc.vector.tensor_copy(ysum[j][:, h, :], py[:])
                        else:
                            nc.vector.tensor_tensor(
                                ysum[j][:, h, :],
                                py[:],
                                ysum[j][:, h, :],
                                mybir.AluOpType.add,
                            )

            def norm_oproj(j, dmaq, dmaq2=None):
                dmaq2 = dmaq2 or dmaq
                # 1/rowsum, broadcast via emat matmul, scale, out-project
                lr = rpool.tile([HPG, 512], F32, name="lr", tag="lr")
                nc.gpsimd.dma_start(out=lr[:], in_=ysum[j][64:65, :, :])
                rinv = rpool.tile([HPG, 512], F32, name="rinv", tag="rinv")
                nc.vector.reciprocal_approx_fast(rinv[:], lr[:])
                rr16 = rpool.tile([HPG, 512], DT, name="rr16", tag="rr16")
                nc.vector.tensor_copy(rr16[:], rinv[:])
                ytiles = []
                for c in range(MC):
                    # assemble head pair into [128, 512] (partition-shifted
                    # copy for the odd head), then scale in place
                    yp = ypool.tile([P, 512], DT, name=f"y{c}", tag=f"y{c}")
                    nc.vector.tensor_copy(
                        yp[0:64, :], ysum[j][0:64, 2 * c, :]
                    )
                    nc.vector.tensor_copy(
                        yp[64:128, :], ysum[j][0:64, 2 * c + 1, :]
                    )
                    pr = psA.tile([P, 512], F32, name="pr", tag="psA")
                    nc.tensor.matmul(
                        pr[:], emat[:, c, :], rr16[:], start=True, stop=True
                    )
                    rbc = rpool.tile([P, 512], F32, name="rbc", tag="rbc")
                    nc.vector.tensor_copy(rbc[:], pr[:])
                    nc.vector.tensor_mul(yp[:], yp[:], rbc[:])
                    ytiles.append(yp)
                for nd in range(2):
                    for mt in range(4):
                        po = psA.tile([P, 512], F32, name="po", tag="psA")
                        for c in range(MC):
                            nc.tensor.matmul(
                                po[:],
                                ytiles[c][:, mt * P : (mt + 1) * P],
                                wp_sb[:, c, nd * 512 : (nd + 1) * 512],
                                start=(c == 0),
                                stop=(c == MC - 1),
                            )
                        ot = opool.tile([P, 512], DT, name="ot", tag="ot")
                        nc.vector.tensor_copy(ot[:], po[:])
                        q = dmaq if nd == 0 else dmaq2
                        q.dma_start(out=out.ap()[j, mt, nd], in_=ot[:])

            # ---- emission ----
            # startup: per-head-pair interleave so every projection that a
            # scores matmul reads is emitted before it (Tile RAW edges come
            # from emission order)
            xq0 = xq_dma(0)
            nc.sync.dma_start(out=wv_sb[:], in_=wvT.ap())
            xk0, xv0 = kv_dma(0)
            qproj_block(0, xq0, ms=(0,))
            kproj_stripe(0, xk0, ms=(0,))
            vproj_stripe(0, xv0)
            attn_stripe_block(0, 0, hps=(0,))
            qproj_block(0, xq0, ms=(1,))
            kproj_stripe(0, xk0, ms=(1,))
            attn_stripe_block(0, 0, hps=(1,))
            qproj_block(0, xq0, ms=(2, 3))
            kproj_stripe(0, xk0, ms=(2, 3))
            attn_stripe_block(0, 0, hps=(2, 3))
            xq1 = xq_dma(1)
            nc.sync.dma_start(out=wp_sb[:], in_=wpg.ap())
            qproj_block(1, xq1)
            xkn, xvn = kv_dma(1)
            attn_stripe_block(0, 1)
            xq2 = xq_dma(2)
            qproj_block(2, xq2)
            attn_stripe_block(0, 2)
            xq3 = xq_dma(3)
            qproj_block(3, xq3)
            attn_stripe_block(0, 3)
            for s in range(1, NJ):
                kproj_stripe(s, xkn)
                vproj_stripe(s, xvn)
                norm_oproj(s - 1, nc.sync)
                for j in range(s, NJ):
                    if j == s + 1 and s + 1 < NJ:
                        xkn, xvn = kv_dma(s + 1)
                    attn_stripe_block(s, j)
            norm_oproj(NJ - 1, nc.sync, nc.gpsimd)

    nc.compile()
    return nc


def kernel(query_data, key_data, value_data, Wq, Wk, Wv, Wp, bp):
    query_data = np.asarray(query_data, dtype=np.float32)
    key_data = np.asarray(key_data, dtype=np.float32)
    value_data = np.asarray(value_data, dtype=np.float32)
    Wq = np.asarray(Wq, dtype=np.float32)
    Wk = np.asarray(Wk, dtype=np.float32)
    Wv = np.asarray(Wv, dtype=np.float32)
    Wp = np.asarray(Wp, dtype=np.float32)
    bp = np.asarray(bp, dtype=np.float32)

    if "nc" not in _CACHE:
        _CACHE["nc"] = _build()
    nc = _CACHE["nc"]

    def _xblk(x):
        # [S, D] -> [NJ, P, KC, 512]: line (p) holds all KC d-chunks of a
        # 512-token block contiguously
        return np.ascontiguousarray(
            x.T.reshape(KC, P, NJ, 512).transpose(2, 1, 0, 3)
        ).astype(NPDT)

    def _wblk(wT, chunks):
        # [D_in, N] -> [P, chunks, N]
        n = wT.shape[1]
        return np.ascontiguousarray(
            wT.reshape(chunks, P, n).transpose(1, 0, 2)
        ).astype(NPDT)

    in_maps = []
    for c in range(NC):
        b, g = divmod(c, G)
        sl = slice(g * HD, (g + 1) * HD)
        in_maps.append(
            {
                "xqT": _xblk(query_data[b]),
                "xkT": _xblk(key_data[b]),
                "xvT": _xblk(value_data[b]),
                "wqT": _wblk(Wq[sl, :].T, KC),
                "wkT": _wblk(Wk[sl, :].T, KC),
                "wvT": _wblk(Wv[sl, :].T, KC),
                "wpg": _wblk(Wp[:, sl].T, MC),
                "ein": _emat(),
            }
        )

    res = run_bass_kernel_spmd(nc, in_maps, core_ids=list(range(NC)))
    _CACHE["last_results"] = res

    out = np.zeros((B, S, D), dtype=np.float32)
    for c in range(NC):
        b = c // G
        o = res.results[c]["out"]  # [NJ, 4, 2, P, 512] fp16
        out[b] += (
            o.transpose(0, 1, 3, 2, 4).reshape(S, D).astype(np.float32)
        )
    out += bp
    return out
